# revision 17
# baseline (speedup 1.0000x reference)
"""Trainium2 Bass kernel for nn_MemoryPlus (retrieval_knn).

Strategy (8 NeuronCores, data-parallel over the 4096 tokens, 512/core):
  sims = q @ k_norm^T runs in plain fp32 on the PE: top-32-of-32768
  selection gaps are ~1.6e-3 relative, so 16-bit (and even fp32r/tf32)
  matmuls mis-select members and blow the 2e-2 error budget; fp32 also
  turns out to be the fastest precise option on this part because the
  PE's utilization throttle duty-cycles 1-cycle/row dtypes to ~50%
  while the 4-pass fp32 mode runs unthrottled.  The top-8 per
  1024-shard scan (max8 + find_index8) likewise runs on fp32 values.
  The 256 candidates reduce to an exact top-32 (max8/match_replace),
  softmax runs on rq-scaled logits, value rows are fetched with gpsimd
  dma_gather (bf16) on two software-DGE queues, and the weighted sum
  runs on the PE as diag(w_j) matmuls accumulating into PSUM (keeps
  the DVE free for the scan; the diag matrices are built in one
  broadcast DVE op per tile).  gate and the output projection run in
  bf16.  Keys are packed host-side into contiguous 512KB blocks so
  each k-block is a single DMA, and token tiles are staggered so tile
  tails (top-32 + gather + accumulate) overlap other tiles' matmuls.

Host-side work is layout only (transposes / normalization / dtype
packing of fixed weights+inputs).
"""

import os

import ml_dtypes
import numpy as np

import concourse.bass as bass
import concourse.tile as tile
from concourse import bacc, mybir
from concourse.bass_utils import run_bass_kernel_spmd
from concourse.masks import make_identity

F32 = mybir.dt.float32
BF16 = mybir.dt.bfloat16
I16 = mybir.dt.int16
U16 = mybir.dt.uint16
AF = mybir.ActivationFunctionType
ALU = mybir.AluOpType

N_CORES = 8
NEG = -1.0e30


class Cfg:
    def __init__(self, n_mem=32768, n_ttiles=4, d_model=1024, d_key=256,
                 d_val=1024, k=32, block=1024, step=6, gjc=4):
        self.n_mem = n_mem
        self.n_ttiles = n_ttiles          # token tiles of 128 per core
        self.T = 128 * n_ttiles           # tokens per core
        self.d_model = d_model
        self.d_key = d_key
        self.d_val = d_val
        self.k = k
        self.block = block                # mem block per k DMA (= shard)
        self.n_blocks = n_mem // block
        self.step = step                  # tile stagger offset in blocks
        self.n_cand = 8 * self.n_blocks   # top-8 per shard
        self.gjc = gjc                    # value-gather j-chunk
        assert self.n_cand >= k and k % 8 == 0


FULL = Cfg()


def build(cfg: Cfg):
    nc = bacc.Bacc("TRN2", target_bir_lowering=False, debug=False,
                   num_devices=N_CORES, num_swdge_queues=2)
    dm, dk, dv, T = cfg.d_model, cfg.d_key, cfg.d_val, cfg.T
    n_dm, n_dk, n_dv = dm // 128, dk // 128, dv // 128

    xT = nc.dram_tensor("xT", [128, n_dm, T], F32, kind="ExternalInput").ap()
    wqT = nc.dram_tensor("wqT", [128, n_dm, dk], F32,
                         kind="ExternalInput").ap()
    kpk = nc.dram_tensor("kpk", [cfg.n_blocks, 128, n_dk, cfg.block],
                         F32, kind="ExternalInput").ap()
    wg = nc.dram_tensor("wg", [128, n_dm, dv], BF16,
                        kind="ExternalInput").ap()
    wo = nc.dram_tensor("wo", [128, n_dv, dm], BF16,
                        kind="ExternalInput").ap()
    vals = nc.dram_tensor("vals", [cfg.n_mem, dv], BF16,
                          kind="ExternalInput").ap()
    shof = nc.dram_tensor("shof", [cfg.n_cand], F32,
                          kind="ExternalInput").ap()
    out = nc.dram_tensor("out", [T, dm], F32, kind="ExternalOutput").ap()
    stage = nc.dram_tensor("stage", [cfg.n_ttiles * cfg.k * 128], I16)
    nrmd = nc.dram_tensor("nrmd", [T], F32)

    with tile.TileContext(nc) as tc:
        _kernel_body(tc, cfg, xT, wqT, kpk, wg, wo, vals, shof, out,
                     stage, nrmd)
    nc.compile()
    return nc


def _kernel_body(tc, cfg, xT, wqT, kpk, wg, wo, vals, shof, out,
                 stage, nrmd):
    nc = tc.nc
    dm, dk, dv, T, K = cfg.d_model, cfg.d_key, cfg.d_val, cfg.T, cfg.k
    n_dm, n_dk, n_dv = dm // 128, dk // 128, dv // 128
    NT = cfg.n_ttiles
    NCD = cfg.n_cand
    NB = cfg.n_blocks
    STEP = cfg.step

    with tc.tile_pool(name="persist", bufs=1) as persist:
        ident = persist.tile([128, 128], F32)
        make_identity(nc, ident)
        identb = persist.tile([128, 128], BF16)
        nc.vector.tensor_copy(identb, ident)

        xT_sb = persist.tile([128, n_dm, T], F32)
        nc.sync.dma_start(out=xT_sb, in_=xT)
        xb_sb = persist.tile([128, n_dm, T], BF16)
        nc.scalar.activation(xb_sb, xT_sb, AF.Copy)
        wg_sb = persist.tile([128, n_dm, dv], BF16)
        nc.sync.dma_start(out=wg_sb, in_=wg)
        wo_sb = persist.tile([128, n_dv, dm], BF16)
        nc.scalar.dma_start(out=wo_sb, in_=wo)
        shof_sb = persist.tile([128, NCD], F32)
        nc.scalar.dma_start(
            out=shof_sb,
            in_=bass.AP(tensor=shof.tensor, offset=0, ap=[[0, 128], [1, NCD]]))

        qT_sb = persist.tile([128, n_dk, T], F32)
        rq = persist.tile([128, NT], F32)
        candV = persist.tile([128, NT, NCD], F32)
        candP = persist.tile([128, NT, NCD], U16)
        gate_sb = persist.tile([128, NT, dv], BF16)

        # ---- phase A: qT = wq^T x (fp32) and rq = 1/|q| ----
        with tc.tile_pool(name="qphase", bufs=1) as qp, \
             tc.tile_pool(name="qps", bufs=2, space="PSUM") as qps:
            wq_sb = qp.tile([128, n_dm, dk], F32, tag="wq")
            nc.sync.dma_start(out=wq_sb, in_=wqT)
            for ckp in range(n_dk):
                ps = qps.tile([128, T], F32, tag="qmm")
                for d in range(n_dm):
                    nc.tensor.matmul(ps,
                                     wq_sb[:, d, 128 * ckp:128 * (ckp + 1)],
                                     xT_sb[:, d, :],
                                     start=(d == 0), stop=(d == n_dm - 1))
                nc.scalar.activation(qT_sb[:, ckp, :], ps, AF.Copy)

            # |q|^2 per token via ones-matmul; DRAM round-trip to [128, NT]
            sq = qp.tile([128, n_dk, T], F32, tag="sq")
            nc.scalar.activation(sq, qT_sb, AF.Square)
            ones = qp.tile([128, 1], F32, tag="ones")
            nc.vector.memset(ones, 1.0)
            psn = qps.tile([1, T], F32, tag="qnrm")
            for ckp in range(n_dk):
                nc.tensor.matmul(psn, ones, sq[:, ckp, :],
                                 start=(ckp == 0), stop=(ckp == n_dk - 1))
            nrm_sb = qp.tile([1, T], F32, tag="nrm")
            nc.scalar.activation(nrm_sb, psn, AF.Copy)
            nc.sync.dma_start(
                out=bass.AP(tensor=nrmd, offset=0, ap=[[1, T]]), in_=nrm_sb)
            nrm2 = qp.tile([128, NT], F32, tag="nrm2")
            nc.sync.dma_start(
                out=nrm2,
                in_=bass.AP(tensor=nrmd, offset=0, ap=[[1, 128], [128, NT]]))
            nrms = qp.tile([128, NT], F32, tag="nrms")
            nc.scalar.activation(nrms, nrm2, AF.Sqrt)
            nc.vector.reciprocal(rq, nrms)

        # ---- phase B: sims + scan, staggered; tails when a tile completes ----
        with tc.tile_pool(name="kbp", bufs=3) as kbp, \
             tc.tile_pool(name="evp", bufs=2) as evp, \
             tc.tile_pool(name="tailp", bufs=1) as tp, \
             tc.tile_pool(name="gathp", bufs=1) as gp, \
             tc.tile_pool(name="gop", bufs=2) as gop, \
             tc.tile_pool(name="simps", bufs=3, space="PSUM") as sps, \
             tc.tile_pool(name="dps", bufs=1, space="PSUM") as dps:
            n_steps = NB + STEP * (NT - 1)
            for s in range(n_steps):
                b = s % NB
                kb = kbp.tile([128, n_dk, cfg.block], F32, tag="kb",
                              name="kb")
                nc.sync.dma_start(out=kb, in_=kpk[b, :, :, :])
                for i in range(NT):
                    if not (STEP * i <= s < STEP * i + NB):
                        continue
                    nch = cfg.block // 512
                    pss = sps.tile([128, cfg.block], F32, tag="sim",
                                   name="sim")
                    for ckp in range(n_dk):
                        qch = qT_sb[:, ckp, 128 * i:128 * (i + 1)]
                        for c2 in range(nch):
                            sl = slice(512 * c2, 512 * (c2 + 1))
                            nc.tensor.matmul(pss[:, sl], qch,
                                             kb[:, ckp, sl],
                                             start=(ckp == 0),
                                             stop=(ckp == n_dk - 1),
                                             skip_group_check=True)
                    ev = evp.tile([128, cfg.block], F32, tag=f"ev{i}",
                                  name=f"ev{i}")
                    nc.scalar.activation(ev, pss, AF.Copy)
                    nc.vector.max(candV[:, i, 8 * b:8 * b + 8], ev)
                    nc.vector.max_index(candP[:, i, 8 * b:8 * b + 8],
                                        candV[:, i, 8 * b:8 * b + 8], ev)
                # gate matmul for tile i mid-window (PE + Scalar + 1 DVE mul)
                for i in range(NT):
                    if s == STEP * i + NB // 2:
                        _gate_tile(tc, cfg, i, xb_sb, wg_sb, gate_sb, sps)
                for i in range(NT):
                    if s == STEP * i + NB - 1:
                        w32 = _tail(tc, cfg, i, candV, candP, shof_sb, rq,
                                    stage, tp)
                        _out_tile(tc, cfg, i, w32, vals, identb, gate_sb,
                                  wo_sb, out, stage, tp, gp, gop, dps)


def _gate_tile(tc, cfg, i, xb_sb, wg_sb, gate_sb, sps):
    nc = tc.nc
    n_dm = cfg.d_model // 128
    psg = sps.tile([128, cfg.d_val], F32, tag="sim", name="psg")
    for d in range(n_dm):
        xch = xb_sb[:, d, 128 * i:128 * (i + 1)]
        for h in range(2):
            sl = slice(512 * h, 512 * (h + 1))
            nc.tensor.matmul(psg[:, sl], xch, wg_sb[:, d, sl],
                             start=(d == 0), stop=(d == n_dm - 1),
                             skip_group_check=True)
    # silu(x) = x * sigmoid(x) exactly, matching the reference
    nc.scalar.activation(gate_sb[:, i, :], psg, AF.Sigmoid)
    nc.vector.tensor_mul(gate_sb[:, i, :], gate_sb[:, i, :], psg)


def _tail(tc, cfg, i, candV, candP, shof_sb, rq, stage, tp):
    """Exact top-32 + softmax weights + gather-index staging for tile i."""
    nc = tc.nc
    K, NCD = cfg.k, cfg.n_cand

    scr = tp.tile([128, NCD], F32, tag="scr", name="scr")
    nc.vector.tensor_copy(scr, candV[:, i, :])
    mx = tp.tile([128, K], F32, tag="mx", name="mx")
    for r in range(K // 8):
        nc.vector.max(mx[:, 8 * r:8 * r + 8], scr)
        if r < K // 8 - 1:
            nc.vector.match_replace(scr, mx[:, 8 * r:8 * r + 8], scr, NEG)
    t1 = mx[:, K - 1:K]

    mask = tp.tile([128, NCD], F32, tag="mask", name="mask")
    nc.vector.tensor_scalar(mask, candV[:, i, :], t1, None, ALU.is_ge)
    pfull = tp.tile([128, NCD], F32, tag="pfull", name="pfull")
    nc.vector.tensor_copy(pfull, candP[:, i, :])
    nc.vector.tensor_add(pfull, pfull, shof_sb)
    pfm = tp.tile([128, NCD], F32, tag="pfm", name="pfm")
    nc.vector.tensor_mul(pfm, pfull, mask)

    g32 = tp.tile([128, K], F32, tag="g32", name="g32")
    for r in range(K // 8):
        nc.vector.max(g32[:, 8 * r:8 * r + 8], pfm)
        if r < K // 8 - 1:
            nc.vector.match_replace(pfm, g32[:, 8 * r:8 * r + 8], pfm, 0.0)
    idx16 = tp.tile([128, K], I16, tag="idx16", name="idx16")
    nc.vector.tensor_scalar(idx16, g32, 1.0, None, ALU.subtract)

    # stage j-major to DRAM immediately -- the gather chain (stage -> wr ->
    # dma_gather) is latency-bound and must start before eqscr/softmax
    nc.scalar.dma_start(
        out=bass.AP(tensor=stage, offset=i * K * 128, ap=[[1, 128], [128, K]]),
        in_=idx16)

    # v32[j] = candV at the slot whose (pos+shard offset) == g32[j]
    eqscr = tp.tile([128, NCD], F32, tag="eqscr", name="eqscr")
    v32 = tp.tile([128, K], F32, tag="v32", name="v32")
    for j in range(K):
        nc.vector.scalar_tensor_tensor(eqscr, pfull, g32[:, j:j + 1],
                                       candV[:, i, :], op0=ALU.is_equal,
                                       op1=ALU.mult,
                                       accum_out=v32[:, j:j + 1])

    # softmax over rq * v32; mx[:,0] is the max logit pre-scale
    bexp = tp.tile([128, 1], F32, tag="bexp", name="bexp")
    nc.vector.scalar_tensor_tensor(bexp, mx[:, 0:1], -1.0, rq[:, i:i + 1],
                                   op0=ALU.mult, op1=ALU.mult)
    e32 = tp.tile([128, K], F32, tag="e32", name="e32")
    ssum = tp.tile([128, 1], F32, tag="ssum", name="ssum")
    nc.scalar.activation(e32, v32, AF.Exp, bias=bexp, scale=rq[:, i:i + 1],
                         accum_out=ssum)
    rs = tp.tile([128, 1], F32, tag="rs", name="rs")
    nc.vector.reciprocal(rs, ssum)
    w32 = tp.tile([128, K], F32, tag=f"w32_{i}", name=f"w32_{i}")
    nc.vector.tensor_scalar(w32, e32, rs, None, ALU.mult)
    return w32


def _out_tile(tc, cfg, i, w32, vals, identb, gate_sb, wo_sb, out, stage,
              tp, gp, gop, dps):
    """Gather + weighted value accumulation (diag(w_j) PE matmuls into
    PSUM), gate multiply, transpose, output matmul."""
    nc = tc.nc
    dm, dv, K = cfg.d_model, cfg.d_val, cfg.k
    n_dv = dv // 128

    wr = tp.tile([128, 8 * K], I16, tag="wr", name="wr", bufs=2)
    for g in range(8):
        nc.scalar.dma_start(
            out=wr[16 * g:16 * (g + 1), :],
            in_=bass.AP(tensor=stage, offset=i * K * 128,
                        ap=[[1, 16], [16, 8 * K]]))

    acc = gop.tile([128, dv], BF16, tag="acc", name="acc")
    for jc in range(K // cfg.gjc):
        vg = gp.tile([128, cfg.gjc, dv], BF16, tag="vg", name="vg", bufs=3)
        nc.gpsimd.dma_gather(
            vg, vals, wr[:, 8 * cfg.gjc * jc:8 * cfg.gjc * (jc + 1)],
            num_idxs=128 * cfg.gjc, num_idxs_reg=128 * cfg.gjc,
            elem_size=dv, queue_num=jc % 2)
        for jj in range(cfg.gjc):
            j = cfg.gjc * jc + jj
            if j == 0:
                nc.vector.tensor_scalar(acc, vg[:, jj, :], w32[:, j:j + 1],
                                        None, ALU.mult)
            else:
                nc.vector.scalar_tensor_tensor(acc, vg[:, jj, :],
                                               w32[:, j:j + 1], acc,
                                               op0=ALU.mult, op1=ALU.add)

    # y = mem * gate (bf16)
    y = gop.tile([128, dv], BF16, tag="y", name="y")
    nc.vector.tensor_mul(y, acc, gate_sb[:, i, :])

    yT = gop.tile([128, n_dv, 128], BF16, tag="yT", name="yT")
    nc.sync.dma_start(out=yT, in_=y, transpose=True)
    out_sb = gop.tile([128, dm], F32, tag="outsb", name="outsb")
    pso = dps.tile([128, dm], F32, tag="m512", name="pso")
    for v in range(n_dv):
        for h in range(2):
            sl = slice(512 * h, 512 * (h + 1))
            nc.tensor.matmul(pso[:, sl], yT[:, v, :],
                             wo_sb[:, v, sl],
                             start=(v == 0), stop=(v == n_dv - 1),
                             skip_group_check=True)
    nc.scalar.activation(out_sb, pso, AF.Copy)
    nc.sync.dma_start(out=out[128 * i:128 * (i + 1), :], in_=out_sb)


# ---------------------------------------------------------------- host side

_CACHE = {}


def _prep(x, keys, values, w_q, w_gate, w_out, cfg):
    dm, dk, dv = cfg.d_model, cfg.d_key, cfg.d_val
    n_dm, n_dk, n_dv = dm // 128, dk // 128, dv // 128
    xf = np.ascontiguousarray(x.reshape(-1, dm)).astype(np.float32)

    norm = np.sqrt((keys.astype(np.float64) ** 2).sum(1, keepdims=True))
    knm = (keys / np.maximum(norm, 1e-12)).astype(np.float32)
    knT = np.ascontiguousarray(knm.T)             # [dk, n_mem]
    r = knT.reshape(n_dk, 128, cfg.n_blocks, cfg.block)
    kpk = np.ascontiguousarray(r.transpose(2, 1, 0, 3))

    wqT = np.ascontiguousarray(w_q.T)             # [dm, dk]
    wqp = np.ascontiguousarray(
        wqT.reshape(n_dm, 128, dk).transpose(1, 0, 2))

    wgT = np.ascontiguousarray(w_gate.T)          # [dm, dv]
    wgp = wgT.astype(ml_dtypes.bfloat16).reshape(n_dm, 128, dv)
    wgp = np.ascontiguousarray(wgp.transpose(1, 0, 2))
    woT = np.ascontiguousarray(w_out.T)           # [dv, dm]
    wop = woT.astype(ml_dtypes.bfloat16).reshape(n_dv, 128, dm)
    wop = np.ascontiguousarray(wop.transpose(1, 0, 2))

    shof = ((np.arange(cfg.n_cand, dtype=np.float32) // 8) * cfg.block
            + 1.0).astype(np.float32)
    common = {
        "kpk": kpk,
        "vals": np.ascontiguousarray(values).astype(ml_dtypes.bfloat16),
        "wqT": wqp,
        "wg": wgp,
        "wo": wop,
        "shof": shof,
    }
    in_maps = []
    for c in range(N_CORES):
        xc = xf[c * cfg.T:(c + 1) * cfg.T]        # [T, dm]
        xTc = np.ascontiguousarray(xc.T)          # [dm, T]
        xp = np.ascontiguousarray(
            xTc.reshape(n_dm, 128, cfg.T).transpose(1, 0, 2))
        m = dict(common)
        m["xT"] = xp
        in_maps.append(m)
    return in_maps


def kernel(x, keys, values, w_q, w_gate, w_out):
    cfg = FULL
    if "nc" not in _CACHE:
        _CACHE["nc"] = build(cfg)
    nc = _CACHE["nc"]
    x = np.asarray(x)
    in_maps = _prep(x, np.asarray(keys), np.asarray(values),
                    np.asarray(w_q), np.asarray(w_gate), np.asarray(w_out),
                    cfg)
    trace = os.environ.get("KERNEL_TRACE", "0") == "1"
    if trace:
        try:
            import ntff_shim
            ntff_shim.install()
        except Exception:
            pass
    res = run_bass_kernel_spmd(nc, in_maps, list(range(N_CORES)), trace=trace)
    if trace:
        _CACHE["exec_time_ns"] = res.exec_time_ns
    outs = [res.results[c]["out"] for c in range(N_CORES)]
    B, S, D = x.shape
    return np.concatenate(outs, axis=0).reshape(B, S, D)


# revision 18
# speedup vs baseline: 1.0401x; 1.0401x over previous
"""Trainium2 Bass kernel for nn_MemoryPlus (retrieval_knn).

Strategy (8 NeuronCores, data-parallel over the 4096 tokens, 512/core):
  sims = q @ k_norm^T runs in plain fp32 on the PE: top-32-of-32768
  selection gaps are ~1.6e-3 relative, so 16-bit (and even fp32r/tf32)
  matmuls mis-select members and blow the 2e-2 error budget; fp32 also
  turns out to be the fastest precise option on this part because the
  PE's utilization throttle duty-cycles 1-cycle/row dtypes to ~50%
  while the 4-pass fp32 mode runs unthrottled.  The top-8 per
  1024-shard scan (max8 + find_index8) likewise runs on fp32 values.
  The 256 candidates reduce to an exact top-32 (max8/match_replace),
  softmax runs on rq-scaled logits, value rows are fetched with gpsimd
  dma_gather (bf16) on two software-DGE queues, and the weighted sum
  runs on the PE as diag(w_j) matmuls accumulating into PSUM (keeps
  the DVE free for the scan; the diag matrices are built in one
  broadcast DVE op per tile).  gate and the output projection run in
  bf16.  Keys are packed host-side into contiguous 512KB blocks so
  each k-block is a single DMA, and token tiles are staggered so tile
  tails (top-32 + gather + accumulate) overlap other tiles' matmuls.

Host-side work is layout only (transposes / normalization / dtype
packing of fixed weights+inputs).
"""

import os

import ml_dtypes
import numpy as np

import concourse.bass as bass
import concourse.tile as tile
from concourse import bacc, mybir
from concourse.bass_utils import run_bass_kernel_spmd
from concourse.masks import make_identity

F32 = mybir.dt.float32
BF16 = mybir.dt.bfloat16
I16 = mybir.dt.int16
U16 = mybir.dt.uint16
AF = mybir.ActivationFunctionType
ALU = mybir.AluOpType

N_CORES = 8
NEG = -1.0e30


class Cfg:
    def __init__(self, n_mem=32768, n_ttiles=4, d_model=1024, d_key=256,
                 d_val=1024, k=32, block=1024, step=4, gjc=4):
        self.n_mem = n_mem
        self.n_ttiles = n_ttiles          # token tiles of 128 per core
        self.T = 128 * n_ttiles           # tokens per core
        self.d_model = d_model
        self.d_key = d_key
        self.d_val = d_val
        self.k = k
        self.block = block                # mem block per k DMA (= shard)
        self.n_blocks = n_mem // block
        self.step = step                  # tile stagger offset in blocks
        self.n_cand = 8 * self.n_blocks   # top-8 per shard
        self.gjc = gjc                    # value-gather j-chunk
        assert self.n_cand >= k and k % 8 == 0


FULL = Cfg()


def build(cfg: Cfg):
    nc = bacc.Bacc("TRN2", target_bir_lowering=False, debug=False,
                   num_devices=N_CORES, num_swdge_queues=2)
    dm, dk, dv, T = cfg.d_model, cfg.d_key, cfg.d_val, cfg.T
    n_dm, n_dk, n_dv = dm // 128, dk // 128, dv // 128

    xT = nc.dram_tensor("xT", [128, n_dm, T], F32, kind="ExternalInput").ap()
    wqT = nc.dram_tensor("wqT", [128, n_dm, dk], F32,
                         kind="ExternalInput").ap()
    kpk = nc.dram_tensor("kpk", [cfg.n_blocks, 128, n_dk, cfg.block],
                         F32, kind="ExternalInput").ap()
    wg = nc.dram_tensor("wg", [128, n_dm, dv], BF16,
                        kind="ExternalInput").ap()
    wo = nc.dram_tensor("wo", [128, n_dv, dm], BF16,
                        kind="ExternalInput").ap()
    vals = nc.dram_tensor("vals", [cfg.n_mem, dv], BF16,
                          kind="ExternalInput").ap()
    shof = nc.dram_tensor("shof", [cfg.n_cand], F32,
                          kind="ExternalInput").ap()
    out = nc.dram_tensor("out", [T, dm], F32, kind="ExternalOutput").ap()
    stage = nc.dram_tensor("stage", [cfg.n_ttiles * cfg.k * 128], I16)
    nrmd = nc.dram_tensor("nrmd", [T], F32)

    with tile.TileContext(nc) as tc:
        _kernel_body(tc, cfg, xT, wqT, kpk, wg, wo, vals, shof, out,
                     stage, nrmd)
    nc.compile()
    return nc


def _kernel_body(tc, cfg, xT, wqT, kpk, wg, wo, vals, shof, out,
                 stage, nrmd):
    nc = tc.nc
    dm, dk, dv, T, K = cfg.d_model, cfg.d_key, cfg.d_val, cfg.T, cfg.k
    n_dm, n_dk, n_dv = dm // 128, dk // 128, dv // 128
    NT = cfg.n_ttiles
    NCD = cfg.n_cand
    NB = cfg.n_blocks
    STEP = cfg.step

    with tc.tile_pool(name="persist", bufs=1) as persist:
        ident = persist.tile([128, 128], F32)
        make_identity(nc, ident)
        identb = persist.tile([128, 128], BF16)
        nc.vector.tensor_copy(identb, ident)

        xT_sb = persist.tile([128, n_dm, T], F32)
        nc.sync.dma_start(out=xT_sb, in_=xT)
        xb_sb = persist.tile([128, n_dm, T], BF16)
        nc.scalar.activation(xb_sb, xT_sb, AF.Copy)
        wg_sb = persist.tile([128, n_dm, dv], BF16)
        nc.sync.dma_start(out=wg_sb, in_=wg)
        wo_sb = persist.tile([128, n_dv, dm], BF16)
        nc.scalar.dma_start(out=wo_sb, in_=wo)
        shof_sb = persist.tile([128, NCD], F32)
        nc.scalar.dma_start(
            out=shof_sb,
            in_=bass.AP(tensor=shof.tensor, offset=0, ap=[[0, 128], [1, NCD]]))

        qT_sb = persist.tile([128, n_dk, T], F32)
        rq = persist.tile([128, NT], F32)
        candV = persist.tile([128, NT, NCD], F32)
        candP = persist.tile([128, NT, NCD], U16)
        gate_sb = persist.tile([128, NT, dv], BF16)

        # ---- phase A: qT = wq^T x (fp32) and rq = 1/|q| ----
        with tc.tile_pool(name="qphase", bufs=1) as qp, \
             tc.tile_pool(name="qps", bufs=2, space="PSUM") as qps:
            wq_sb = qp.tile([128, n_dm, dk], F32, tag="wq")
            nc.sync.dma_start(out=wq_sb, in_=wqT)
            for ckp in range(n_dk):
                ps = qps.tile([128, T], F32, tag="qmm")
                for d in range(n_dm):
                    nc.tensor.matmul(ps,
                                     wq_sb[:, d, 128 * ckp:128 * (ckp + 1)],
                                     xT_sb[:, d, :],
                                     start=(d == 0), stop=(d == n_dm - 1))
                nc.scalar.activation(qT_sb[:, ckp, :], ps, AF.Copy)

            # |q|^2 per token via ones-matmul; DRAM round-trip to [128, NT]
            sq = qp.tile([128, n_dk, T], F32, tag="sq")
            nc.scalar.activation(sq, qT_sb, AF.Square)
            ones = qp.tile([128, 1], F32, tag="ones")
            nc.vector.memset(ones, 1.0)
            psn = qps.tile([1, T], F32, tag="qnrm")
            for ckp in range(n_dk):
                nc.tensor.matmul(psn, ones, sq[:, ckp, :],
                                 start=(ckp == 0), stop=(ckp == n_dk - 1))
            nrm_sb = qp.tile([1, T], F32, tag="nrm")
            nc.scalar.activation(nrm_sb, psn, AF.Copy)
            nc.sync.dma_start(
                out=bass.AP(tensor=nrmd, offset=0, ap=[[1, T]]), in_=nrm_sb)
            nrm2 = qp.tile([128, NT], F32, tag="nrm2")
            nc.sync.dma_start(
                out=nrm2,
                in_=bass.AP(tensor=nrmd, offset=0, ap=[[1, 128], [128, NT]]))
            nrms = qp.tile([128, NT], F32, tag="nrms")
            nc.scalar.activation(nrms, nrm2, AF.Sqrt)
            nc.vector.reciprocal(rq, nrms)

        # ---- phase B: sims + scan, staggered; tails when a tile completes ----
        with tc.tile_pool(name="kbp", bufs=3) as kbp, \
             tc.tile_pool(name="evp", bufs=2) as evp, \
             tc.tile_pool(name="tailp", bufs=1) as tp, \
             tc.tile_pool(name="gathp", bufs=1) as gp, \
             tc.tile_pool(name="gop", bufs=2) as gop, \
             tc.tile_pool(name="simps", bufs=3, space="PSUM") as sps, \
             tc.tile_pool(name="dps", bufs=1, space="PSUM") as dps:
            n_steps = NB + STEP * (NT - 1)
            for s in range(n_steps):
                b = s % NB
                kb = kbp.tile([128, n_dk, cfg.block], F32, tag="kb",
                              name="kb")
                nc.sync.dma_start(out=kb, in_=kpk[b, :, :, :])
                for i in range(NT):
                    if not (STEP * i <= s < STEP * i + NB):
                        continue
                    nch = cfg.block // 512
                    pss = sps.tile([128, cfg.block], F32, tag="sim",
                                   name="sim")
                    for ckp in range(n_dk):
                        qch = qT_sb[:, ckp, 128 * i:128 * (i + 1)]
                        for c2 in range(nch):
                            sl = slice(512 * c2, 512 * (c2 + 1))
                            nc.tensor.matmul(pss[:, sl], qch,
                                             kb[:, ckp, sl],
                                             start=(ckp == 0),
                                             stop=(ckp == n_dk - 1),
                                             skip_group_check=True)
                    nc.vector.max(candV[:, i, 8 * b:8 * b + 8], pss)
                    nc.vector.max_index(candP[:, i, 8 * b:8 * b + 8],
                                        candV[:, i, 8 * b:8 * b + 8], pss)
                # gate matmul for tile i mid-window (PE + Scalar + 1 DVE mul)
                for i in range(NT):
                    if s == STEP * i + NB // 2:
                        _gate_tile(tc, cfg, i, xb_sb, wg_sb, gate_sb, sps)
                for i in range(NT):
                    if s == STEP * i + NB - 1:
                        with tc.high_priority():
                            w32 = _tail(tc, cfg, i, candV, candP, shof_sb,
                                        rq, stage, tp)
                            _out_tile(tc, cfg, i, w32, vals, identb,
                                      gate_sb, wo_sb, out, stage, tp, gp,
                                      gop, dps)


def _gate_tile(tc, cfg, i, xb_sb, wg_sb, gate_sb, sps):
    nc = tc.nc
    n_dm = cfg.d_model // 128
    psg = sps.tile([128, cfg.d_val], F32, tag="sim", name="psg")
    for d in range(n_dm):
        xch = xb_sb[:, d, 128 * i:128 * (i + 1)]
        for h in range(2):
            sl = slice(512 * h, 512 * (h + 1))
            nc.tensor.matmul(psg[:, sl], xch, wg_sb[:, d, sl],
                             start=(d == 0), stop=(d == n_dm - 1),
                             skip_group_check=True)
    # silu(x) = x * sigmoid(x) exactly, matching the reference
    nc.scalar.activation(gate_sb[:, i, :], psg, AF.Sigmoid)
    nc.vector.tensor_mul(gate_sb[:, i, :], gate_sb[:, i, :], psg)


def _tail(tc, cfg, i, candV, candP, shof_sb, rq, stage, tp):
    """Exact top-32 + softmax weights + gather-index staging for tile i."""
    nc = tc.nc
    K, NCD = cfg.k, cfg.n_cand

    scr = tp.tile([128, NCD], F32, tag="scr", name="scr")
    nc.vector.tensor_copy(scr, candV[:, i, :])
    mx = tp.tile([128, K], F32, tag="mx", name="mx")
    for r in range(K // 8):
        nc.vector.max(mx[:, 8 * r:8 * r + 8], scr)
        if r < K // 8 - 1:
            nc.vector.match_replace(scr, mx[:, 8 * r:8 * r + 8], scr, NEG)
    t1 = mx[:, K - 1:K]

    mask = tp.tile([128, NCD], F32, tag="mask", name="mask")
    nc.vector.tensor_scalar(mask, candV[:, i, :], t1, None, ALU.is_ge)
    pfull = tp.tile([128, NCD], F32, tag="pfull", name="pfull")
    nc.vector.tensor_copy(pfull, candP[:, i, :])
    nc.vector.tensor_add(pfull, pfull, shof_sb)
    pfm = tp.tile([128, NCD], F32, tag="pfm", name="pfm")
    nc.vector.tensor_mul(pfm, pfull, mask)

    g32 = tp.tile([128, K], F32, tag="g32", name="g32")
    for r in range(K // 8):
        nc.vector.max(g32[:, 8 * r:8 * r + 8], pfm)
        if r < K // 8 - 1:
            nc.vector.match_replace(pfm, g32[:, 8 * r:8 * r + 8], pfm, 0.0)
    idx16 = tp.tile([128, K], I16, tag="idx16", name="idx16")
    nc.vector.tensor_scalar(idx16, g32, 1.0, None, ALU.subtract)

    # stage j-major to DRAM immediately -- the gather chain (stage -> wr ->
    # dma_gather) is latency-bound and must start before eqscr/softmax
    nc.scalar.dma_start(
        out=bass.AP(tensor=stage, offset=i * K * 128, ap=[[1, 128], [128, K]]),
        in_=idx16)

    # v32[j] = candV at the slot whose (pos+shard offset) == g32[j]
    eqscr = tp.tile([128, NCD], F32, tag="eqscr", name="eqscr")
    v32 = tp.tile([128, K], F32, tag="v32", name="v32")
    for j in range(K):
        nc.vector.scalar_tensor_tensor(eqscr, pfull, g32[:, j:j + 1],
                                       candV[:, i, :], op0=ALU.is_equal,
                                       op1=ALU.mult,
                                       accum_out=v32[:, j:j + 1])

    # softmax over rq * v32; mx[:,0] is the max logit pre-scale
    bexp = tp.tile([128, 1], F32, tag="bexp", name="bexp")
    nc.vector.scalar_tensor_tensor(bexp, mx[:, 0:1], -1.0, rq[:, i:i + 1],
                                   op0=ALU.mult, op1=ALU.mult)
    e32 = tp.tile([128, K], F32, tag="e32", name="e32")
    ssum = tp.tile([128, 1], F32, tag="ssum", name="ssum")
    nc.scalar.activation(e32, v32, AF.Exp, bias=bexp, scale=rq[:, i:i + 1],
                         accum_out=ssum)
    rs = tp.tile([128, 1], F32, tag="rs", name="rs")
    nc.vector.reciprocal(rs, ssum)
    w32 = tp.tile([128, K], F32, tag=f"w32_{i}", name=f"w32_{i}")
    nc.vector.tensor_scalar(w32, e32, rs, None, ALU.mult)
    return w32


def _out_tile(tc, cfg, i, w32, vals, identb, gate_sb, wo_sb, out, stage,
              tp, gp, gop, dps):
    """Gather + weighted value accumulation (diag(w_j) PE matmuls into
    PSUM), gate multiply, transpose, output matmul."""
    nc = tc.nc
    dm, dv, K = cfg.d_model, cfg.d_val, cfg.k
    n_dv = dv // 128

    wr = tp.tile([128, 8 * K], I16, tag="wr", name="wr", bufs=2)
    for g in range(8):
        nc.scalar.dma_start(
            out=wr[16 * g:16 * (g + 1), :],
            in_=bass.AP(tensor=stage, offset=i * K * 128,
                        ap=[[1, 16], [16, 8 * K]]))

    psm = dps.tile([128, dv], F32, tag="m512", name="psm")
    for jc in range(K // cfg.gjc):
        vg = gp.tile([128, cfg.gjc, dv], BF16, tag="vg", name="vg", bufs=3)
        nc.gpsimd.dma_gather(
            vg, vals, wr[:, 8 * cfg.gjc * jc:8 * cfg.gjc * (jc + 1)],
            num_idxs=128 * cfg.gjc, num_idxs_reg=128 * cfg.gjc,
            elem_size=dv, queue_num=jc % 2)
        for jj in range(cfg.gjc):
            j = cfg.gjc * jc + jj
            svg = gop.tile([128, dv], BF16, tag="svg", name="svg", bufs=3)
            nc.scalar.activation(svg, vg[:, jj, :], AF.Copy,
                                 scale=w32[:, j:j + 1])
            for h in range(2):
                sl = slice(512 * h, 512 * (h + 1))
                nc.tensor.matmul(psm[:, sl], identb, svg[:, sl],
                                 start=(j == 0), stop=(j == K - 1),
                                 skip_group_check=True)

    # y = mem * gate (bf16), reading mem straight out of PSUM
    y = gop.tile([128, dv], BF16, tag="y", name="y")
    nc.vector.tensor_mul(y, psm, gate_sb[:, i, :])

    yT = gop.tile([128, n_dv, 128], BF16, tag="yT", name="yT")
    nc.sync.dma_start(out=yT, in_=y, transpose=True)
    out_sb = gop.tile([128, dm], F32, tag="outsb", name="outsb")
    pso = dps.tile([128, dm], F32, tag="m512", name="pso")
    for v in range(n_dv):
        for h in range(2):
            sl = slice(512 * h, 512 * (h + 1))
            nc.tensor.matmul(pso[:, sl], yT[:, v, :],
                             wo_sb[:, v, sl],
                             start=(v == 0), stop=(v == n_dv - 1),
                             skip_group_check=True)
    nc.scalar.activation(out_sb, pso, AF.Copy)
    nc.sync.dma_start(out=out[128 * i:128 * (i + 1), :], in_=out_sb)


# ---------------------------------------------------------------- host side

_CACHE = {}


def _prep(x, keys, values, w_q, w_gate, w_out, cfg):
    dm, dk, dv = cfg.d_model, cfg.d_key, cfg.d_val
    n_dm, n_dk, n_dv = dm // 128, dk // 128, dv // 128
    xf = np.ascontiguousarray(x.reshape(-1, dm)).astype(np.float32)

    norm = np.sqrt((keys.astype(np.float64) ** 2).sum(1, keepdims=True))
    knm = (keys / np.maximum(norm, 1e-12)).astype(np.float32)
    knT = np.ascontiguousarray(knm.T)             # [dk, n_mem]
    r = knT.reshape(n_dk, 128, cfg.n_blocks, cfg.block)
    kpk = np.ascontiguousarray(r.transpose(2, 1, 0, 3))

    wqT = np.ascontiguousarray(w_q.T)             # [dm, dk]
    wqp = np.ascontiguousarray(
        wqT.reshape(n_dm, 128, dk).transpose(1, 0, 2))

    wgT = np.ascontiguousarray(w_gate.T)          # [dm, dv]
    wgp = wgT.astype(ml_dtypes.bfloat16).reshape(n_dm, 128, dv)
    wgp = np.ascontiguousarray(wgp.transpose(1, 0, 2))
    woT = np.ascontiguousarray(w_out.T)           # [dv, dm]
    wop = woT.astype(ml_dtypes.bfloat16).reshape(n_dv, 128, dm)
    wop = np.ascontiguousarray(wop.transpose(1, 0, 2))

    shof = ((np.arange(cfg.n_cand, dtype=np.float32) // 8) * cfg.block
            + 1.0).astype(np.float32)
    common = {
        "kpk": kpk,
        "vals": np.ascontiguousarray(values).astype(ml_dtypes.bfloat16),
        "wqT": wqp,
        "wg": wgp,
        "wo": wop,
        "shof": shof,
    }
    in_maps = []
    for c in range(N_CORES):
        xc = xf[c * cfg.T:(c + 1) * cfg.T]        # [T, dm]
        xTc = np.ascontiguousarray(xc.T)          # [dm, T]
        xp = np.ascontiguousarray(
            xTc.reshape(n_dm, 128, cfg.T).transpose(1, 0, 2))
        m = dict(common)
        m["xT"] = xp
        in_maps.append(m)
    return in_maps


def kernel(x, keys, values, w_q, w_gate, w_out):
    cfg = FULL
    if "nc" not in _CACHE:
        _CACHE["nc"] = build(cfg)
    nc = _CACHE["nc"]
    x = np.asarray(x)
    in_maps = _prep(x, np.asarray(keys), np.asarray(values),
                    np.asarray(w_q), np.asarray(w_gate), np.asarray(w_out),
                    cfg)
    trace = os.environ.get("KERNEL_TRACE", "0") == "1"
    if trace:
        try:
            import ntff_shim
            ntff_shim.install()
        except Exception:
            pass
    res = run_bass_kernel_spmd(nc, in_maps, list(range(N_CORES)), trace=trace)
    if trace:
        _CACHE["exec_time_ns"] = res.exec_time_ns
    outs = [res.results[c]["out"] for c in range(N_CORES)]
    B, S, D = x.shape
    return np.concatenate(outs, axis=0).reshape(B, S, D)


# revision 20
# speedup vs baseline: 1.2730x; 1.2239x over previous
"""Trainium2 Bass kernel for nn_MemoryPlus (retrieval_knn).

Strategy (8 NeuronCores, data-parallel over the 4096 tokens, 512/core):
  sims = q @ k_norm^T runs in plain fp32 on the PE: top-32-of-32768
  selection gaps are ~1.6e-3 relative, so 16-bit (and even fp32r/tf32)
  matmuls mis-select members and blow the 2e-2 error budget; fp32 also
  turns out to be the fastest precise option on this part because the
  PE's utilization throttle duty-cycles 1-cycle/row dtypes to ~50%
  while the 4-pass fp32 mode runs unthrottled.  The top-8 per
  1024-shard scan (max8 + find_index8) likewise runs on fp32 values.
  The 256 candidates reduce to an exact top-32 (max8/match_replace),
  softmax runs on rq-scaled logits, value rows are fetched with gpsimd
  dma_gather (bf16) on two software-DGE queues, and the weighted sum
  runs on the PE as diag(w_j) matmuls accumulating into PSUM (keeps
  the DVE free for the scan; the diag matrices are built in one
  broadcast DVE op per tile).  gate and the output projection run in
  bf16.  Keys are packed host-side into contiguous 512KB blocks so
  each k-block is a single DMA, and token tiles are staggered so tile
  tails (top-32 + gather + accumulate) overlap other tiles' matmuls.

Host-side work is layout only (transposes / normalization / dtype
packing of fixed weights+inputs).
"""

import os

import ml_dtypes
import numpy as np

import concourse.bass as bass
import concourse.tile as tile
from concourse import bacc, mybir
from concourse.bass_utils import run_bass_kernel_spmd
from concourse.masks import make_identity

F32 = mybir.dt.float32
BF16 = mybir.dt.bfloat16
I16 = mybir.dt.int16
U16 = mybir.dt.uint16
AF = mybir.ActivationFunctionType
ALU = mybir.AluOpType

N_CORES = 8
NEG = -1.0e30


class Cfg:
    def __init__(self, n_mem=32768, n_ttiles=4, d_model=1024, d_key=256,
                 d_val=1024, k=32, block=1024, step=4, gjc=4):
        self.n_mem = n_mem
        self.n_ttiles = n_ttiles          # token tiles of 128 per core
        self.T = 128 * n_ttiles           # tokens per core
        self.d_model = d_model
        self.d_key = d_key
        self.d_val = d_val
        self.k = k
        self.block = block                # mem block per k DMA (= shard)
        self.n_blocks = n_mem // block
        self.step = step                  # tile stagger offset in blocks
        self.n_cand = 8 * self.n_blocks   # top-8 per shard
        self.gjc = gjc                    # value-gather j-chunk
        assert self.n_cand >= k and k % 8 == 0


FULL = Cfg()


def build(cfg: Cfg):
    nc = bacc.Bacc("TRN2", target_bir_lowering=False, debug=False,
                   num_devices=N_CORES, num_swdge_queues=2)
    dm, dk, dv, T = cfg.d_model, cfg.d_key, cfg.d_val, cfg.T
    n_dm, n_dk, n_dv = dm // 128, dk // 128, dv // 128

    xT = nc.dram_tensor("xT", [128, n_dm, T], F32, kind="ExternalInput").ap()
    wqT = nc.dram_tensor("wqT", [128, n_dm, dk], F32,
                         kind="ExternalInput").ap()
    kpk = nc.dram_tensor("kpk", [cfg.n_blocks, 128, n_dk, cfg.block],
                         F32, kind="ExternalInput").ap()
    wg = nc.dram_tensor("wg", [128, n_dm, dv], BF16,
                        kind="ExternalInput").ap()
    wo = nc.dram_tensor("wo", [128, n_dv, dm], BF16,
                        kind="ExternalInput").ap()
    vals = nc.dram_tensor("vals", [cfg.n_mem, dv], BF16,
                          kind="ExternalInput").ap()
    shof = nc.dram_tensor("shof", [cfg.n_cand], F32,
                          kind="ExternalInput").ap()
    out = nc.dram_tensor("out", [T, dm], F32, kind="ExternalOutput").ap()
    stage = nc.dram_tensor("stage", [cfg.n_ttiles * cfg.k * 128], I16)
    nrmd = nc.dram_tensor("nrmd", [T], F32)

    with tile.TileContext(nc) as tc:
        _kernel_body(tc, cfg, xT, wqT, kpk, wg, wo, vals, shof, out,
                     stage, nrmd)
    nc.compile()
    return nc


def _kernel_body(tc, cfg, xT, wqT, kpk, wg, wo, vals, shof, out,
                 stage, nrmd):
    nc = tc.nc
    dm, dk, dv, T, K = cfg.d_model, cfg.d_key, cfg.d_val, cfg.T, cfg.k
    n_dm, n_dk, n_dv = dm // 128, dk // 128, dv // 128
    NT = cfg.n_ttiles
    NCD = cfg.n_cand
    NB = cfg.n_blocks
    STEP = cfg.step

    with tc.tile_pool(name="persist", bufs=1) as persist:
        ident = persist.tile([128, 128], F32)
        make_identity(nc, ident)
        identb = persist.tile([128, 128], BF16)
        nc.vector.tensor_copy(identb, ident)

        xT_sb = persist.tile([128, n_dm, T], F32)
        nc.sync.dma_start(out=xT_sb, in_=xT)
        xb_sb = persist.tile([128, n_dm, T], BF16)
        nc.scalar.activation(xb_sb, xT_sb, AF.Copy)
        wg_sb = persist.tile([128, n_dm, dv], BF16)
        nc.sync.dma_start(out=wg_sb, in_=wg)
        wo_sb = persist.tile([128, n_dv, dm], BF16)
        nc.scalar.dma_start(out=wo_sb, in_=wo)
        shof_sb = persist.tile([128, NCD], F32)
        nc.scalar.dma_start(
            out=shof_sb,
            in_=bass.AP(tensor=shof.tensor, offset=0, ap=[[0, 128], [1, NCD]]))

        qT_sb = persist.tile([128, n_dk, T], F32)
        rq = persist.tile([128, NT], F32)
        candV = persist.tile([128, NT, NCD], F32)
        candP = persist.tile([128, NT, NCD], U16)
        gate_sb = persist.tile([128, NT, dv], BF16)

        # ---- phase A: qT = wq^T x (fp32) and rq = 1/|q| ----
        with tc.tile_pool(name="qphase", bufs=1) as qp, \
             tc.tile_pool(name="qps", bufs=2, space="PSUM") as qps:
            wq_sb = qp.tile([128, n_dm, dk], F32, tag="wq")
            nc.sync.dma_start(out=wq_sb, in_=wqT)
            for ckp in range(n_dk):
                ps = qps.tile([128, T], F32, tag="qmm")
                for d in range(n_dm):
                    nc.tensor.matmul(ps,
                                     wq_sb[:, d, 128 * ckp:128 * (ckp + 1)],
                                     xT_sb[:, d, :],
                                     start=(d == 0), stop=(d == n_dm - 1))
                nc.scalar.activation(qT_sb[:, ckp, :], ps, AF.Copy)

            # |q|^2 per token via ones-matmul; DRAM round-trip to [128, NT]
            sq = qp.tile([128, n_dk, T], F32, tag="sq")
            nc.scalar.activation(sq, qT_sb, AF.Square)
            ones = qp.tile([128, 1], F32, tag="ones")
            nc.vector.memset(ones, 1.0)
            psn = qps.tile([1, T], F32, tag="qnrm")
            for ckp in range(n_dk):
                nc.tensor.matmul(psn, ones, sq[:, ckp, :],
                                 start=(ckp == 0), stop=(ckp == n_dk - 1))
            nrm_sb = qp.tile([1, T], F32, tag="nrm")
            nc.scalar.activation(nrm_sb, psn, AF.Copy)
            nc.sync.dma_start(
                out=bass.AP(tensor=nrmd, offset=0, ap=[[1, T]]), in_=nrm_sb)
            nrm2 = qp.tile([128, NT], F32, tag="nrm2")
            nc.sync.dma_start(
                out=nrm2,
                in_=bass.AP(tensor=nrmd, offset=0, ap=[[1, 128], [128, NT]]))
            nrms = qp.tile([128, NT], F32, tag="nrms")
            nc.scalar.activation(nrms, nrm2, AF.Sqrt)
            nc.vector.reciprocal(rq, nrms)

        # ---- phase B: sims + scan, staggered; tails when a tile completes ----
        with tc.tile_pool(name="kbp", bufs=3) as kbp, \
             tc.tile_pool(name="evp", bufs=2) as evp, \
             tc.tile_pool(name="tailp", bufs=1) as tp, \
             tc.tile_pool(name="gathp", bufs=1) as gp, \
             tc.tile_pool(name="gop", bufs=2) as gop, \
             tc.tile_pool(name="simps", bufs=3, space="PSUM") as sps, \
             tc.tile_pool(name="dps", bufs=1, space="PSUM") as dps:
            for s in range(2 * NB):
                b = s % NB
                pair = s // NB
                kb = kbp.tile([128, n_dk, cfg.block], F32, tag="kb",
                              name="kb")
                nc.sync.dma_start(out=kb, in_=kpk[b, :, :, :])
                for i in (2 * pair, 2 * pair + 1):
                    nch = cfg.block // 512
                    pss = sps.tile([128, cfg.block], F32, tag="sim",
                                   name="sim")
                    for ckp in range(n_dk):
                        qch = qT_sb[:, ckp, 128 * i:128 * (i + 1)]
                        for c2 in range(nch):
                            sl = slice(512 * c2, 512 * (c2 + 1))
                            nc.tensor.matmul(pss[:, sl], qch,
                                             kb[:, ckp, sl],
                                             start=(ckp == 0),
                                             stop=(ckp == n_dk - 1),
                                             skip_group_check=True)
                    nc.vector.max(candV[:, i, 8 * b:8 * b + 8], pss)
                    nc.vector.max_index(candP[:, i, 8 * b:8 * b + 8],
                                        candV[:, i, 8 * b:8 * b + 8], pss)
                # gate matmul mid-window (PE + Scalar + 1 DVE mul)
                if b == NB // 2:
                    for i in (2 * pair, 2 * pair + 1):
                        _gate_tile(tc, cfg, i, xb_sb, wg_sb, gate_sb, sps)
                if b == NB - 1:
                    for i in (2 * pair, 2 * pair + 1):
                        with tc.high_priority():
                            w32 = _tail(tc, cfg, i, candV, candP, shof_sb,
                                        rq, stage, tp)
                            _out_tile(tc, cfg, i, w32, vals, identb,
                                      gate_sb, wo_sb, out, stage, tp, gp,
                                      gop, dps)


def _gate_tile(tc, cfg, i, xb_sb, wg_sb, gate_sb, sps):
    nc = tc.nc
    n_dm = cfg.d_model // 128
    psg = sps.tile([128, cfg.d_val], F32, tag="sim", name="psg")
    for d in range(n_dm):
        xch = xb_sb[:, d, 128 * i:128 * (i + 1)]
        for h in range(2):
            sl = slice(512 * h, 512 * (h + 1))
            nc.tensor.matmul(psg[:, sl], xch, wg_sb[:, d, sl],
                             start=(d == 0), stop=(d == n_dm - 1),
                             skip_group_check=True)
    # silu(x) = x * sigmoid(x) exactly, matching the reference
    nc.scalar.activation(gate_sb[:, i, :], psg, AF.Sigmoid)
    nc.vector.tensor_mul(gate_sb[:, i, :], gate_sb[:, i, :], psg)


def _tail(tc, cfg, i, candV, candP, shof_sb, rq, stage, tp):
    """Exact top-32 + softmax weights + gather-index staging for tile i."""
    nc = tc.nc
    K, NCD = cfg.k, cfg.n_cand

    scr = tp.tile([128, NCD], F32, tag="scr", name="scr")
    nc.vector.tensor_copy(scr, candV[:, i, :])
    mx = tp.tile([128, K], F32, tag="mx", name="mx")
    for r in range(K // 8):
        nc.vector.max(mx[:, 8 * r:8 * r + 8], scr)
        if r < K // 8 - 1:
            nc.vector.match_replace(scr, mx[:, 8 * r:8 * r + 8], scr, NEG)
    t1 = mx[:, K - 1:K]

    mask = tp.tile([128, NCD], F32, tag="mask", name="mask")
    nc.vector.tensor_scalar(mask, candV[:, i, :], t1, None, ALU.is_ge)
    pfull = tp.tile([128, NCD], F32, tag="pfull", name="pfull")
    nc.vector.tensor_copy(pfull, candP[:, i, :])
    nc.vector.tensor_add(pfull, pfull, shof_sb)
    pfm = tp.tile([128, NCD], F32, tag="pfm", name="pfm")
    nc.vector.tensor_mul(pfm, pfull, mask)

    g32 = tp.tile([128, K], F32, tag="g32", name="g32")
    for r in range(K // 8):
        nc.vector.max(g32[:, 8 * r:8 * r + 8], pfm)
        if r < K // 8 - 1:
            nc.vector.match_replace(pfm, g32[:, 8 * r:8 * r + 8], pfm, 0.0)
    idx16 = tp.tile([128, K], I16, tag="idx16", name="idx16")
    nc.vector.tensor_scalar(idx16, g32, 1.0, None, ALU.subtract)

    # stage j-major to DRAM immediately -- the gather chain (stage -> wr ->
    # dma_gather) is latency-bound and must start before eqscr/softmax
    # write pre-wrapped: stage[(p%16)*256 + (p//16) + 8*j] = idx16[p, j],
    # so the gather-index readback is contiguous 512B per partition
    nc.sync.dma_start(
        out=bass.AP(tensor=stage, offset=i * K * 128,
                    ap=[[1, 8], [8 * K, 16], [8, K]]),
        in_=idx16)

    # v32[j] = candV at the slot whose (pos+shard offset) == g32[j]
    eqscr = tp.tile([128, NCD], F32, tag="eqscr", name="eqscr")
    v32 = tp.tile([128, K], F32, tag="v32", name="v32")
    for j in range(K):
        nc.vector.scalar_tensor_tensor(eqscr, pfull, g32[:, j:j + 1],
                                       candV[:, i, :], op0=ALU.is_equal,
                                       op1=ALU.mult,
                                       accum_out=v32[:, j:j + 1])

    # softmax over rq * v32; mx[:,0] is the max logit pre-scale
    bexp = tp.tile([128, 1], F32, tag="bexp", name="bexp")
    nc.vector.scalar_tensor_tensor(bexp, mx[:, 0:1], -1.0, rq[:, i:i + 1],
                                   op0=ALU.mult, op1=ALU.mult)
    e32 = tp.tile([128, K], F32, tag="e32", name="e32")
    ssum = tp.tile([128, 1], F32, tag="ssum", name="ssum")
    nc.scalar.activation(e32, v32, AF.Exp, bias=bexp, scale=rq[:, i:i + 1],
                         accum_out=ssum)
    rs = tp.tile([128, 1], F32, tag="rs", name="rs")
    nc.vector.reciprocal(rs, ssum)
    w32 = tp.tile([128, K], F32, tag=f"w32_{i}", name=f"w32_{i}")
    nc.vector.tensor_scalar(w32, e32, rs, None, ALU.mult)
    return w32


def _out_tile(tc, cfg, i, w32, vals, identb, gate_sb, wo_sb, out, stage,
              tp, gp, gop, dps):
    """Gather + weighted value accumulation (diag(w_j) PE matmuls into
    PSUM), gate multiply, transpose, output matmul."""
    nc = tc.nc
    dm, dv, K = cfg.d_model, cfg.d_val, cfg.k
    n_dv = dv // 128

    wr = tp.tile([128, 8 * K], I16, tag="wr", name="wr", bufs=2)
    nc.sync.dma_start(
        out=wr,
        in_=bass.AP(tensor=stage, offset=i * K * 128,
                    ap=[[0, 8], [8 * K, 16], [1, 8 * K]]))

    psm = dps.tile([128, dv], F32, tag="m512", name="psm")
    for jc in range(K // cfg.gjc):
        vg = gp.tile([128, cfg.gjc, dv], BF16, tag="vg", name="vg", bufs=3)
        nc.gpsimd.dma_gather(
            vg, vals, wr[:, 8 * cfg.gjc * jc:8 * cfg.gjc * (jc + 1)],
            num_idxs=128 * cfg.gjc, num_idxs_reg=128 * cfg.gjc,
            elem_size=dv, queue_num=jc % 2)
        for jj in range(cfg.gjc):
            j = cfg.gjc * jc + jj
            svg = gop.tile([128, dv], BF16, tag="svg", name="svg", bufs=3)
            nc.scalar.activation(svg, vg[:, jj, :], AF.Copy,
                                 scale=w32[:, j:j + 1])
            for h in range(2):
                sl = slice(512 * h, 512 * (h + 1))
                nc.tensor.matmul(psm[:, sl], identb, svg[:, sl],
                                 start=(j == 0), stop=(j == K - 1),
                                 skip_group_check=True)

    # y = mem * gate (bf16), reading mem straight out of PSUM
    y = gop.tile([128, dv], BF16, tag="y", name="y")
    nc.vector.tensor_mul(y, psm, gate_sb[:, i, :])

    yT = gop.tile([128, n_dv, 128], BF16, tag="yT", name="yT")
    nc.sync.dma_start(out=yT, in_=y, transpose=True)
    out_sb = gop.tile([128, dm], F32, tag="outsb", name="outsb")
    pso = dps.tile([128, dm], F32, tag="m512", name="pso")
    for v in range(n_dv):
        for h in range(2):
            sl = slice(512 * h, 512 * (h + 1))
            nc.tensor.matmul(pso[:, sl], yT[:, v, :],
                             wo_sb[:, v, sl],
                             start=(v == 0), stop=(v == n_dv - 1),
                             skip_group_check=True)
    nc.scalar.activation(out_sb, pso, AF.Copy)
    nc.sync.dma_start(out=out[128 * i:128 * (i + 1), :], in_=out_sb)


# ---------------------------------------------------------------- host side

_CACHE = {}


def _prep(x, keys, values, w_q, w_gate, w_out, cfg):
    dm, dk, dv = cfg.d_model, cfg.d_key, cfg.d_val
    n_dm, n_dk, n_dv = dm // 128, dk // 128, dv // 128
    xf = np.ascontiguousarray(x.reshape(-1, dm)).astype(np.float32)

    norm = np.sqrt((keys.astype(np.float64) ** 2).sum(1, keepdims=True))
    knm = (keys / np.maximum(norm, 1e-12)).astype(np.float32)
    knT = np.ascontiguousarray(knm.T)             # [dk, n_mem]
    r = knT.reshape(n_dk, 128, cfg.n_blocks, cfg.block)
    kpk = np.ascontiguousarray(r.transpose(2, 1, 0, 3))

    wqT = np.ascontiguousarray(w_q.T)             # [dm, dk]
    wqp = np.ascontiguousarray(
        wqT.reshape(n_dm, 128, dk).transpose(1, 0, 2))

    wgT = np.ascontiguousarray(w_gate.T)          # [dm, dv]
    wgp = wgT.astype(ml_dtypes.bfloat16).reshape(n_dm, 128, dv)
    wgp = np.ascontiguousarray(wgp.transpose(1, 0, 2))
    woT = np.ascontiguousarray(w_out.T)           # [dv, dm]
    wop = woT.astype(ml_dtypes.bfloat16).reshape(n_dv, 128, dm)
    wop = np.ascontiguousarray(wop.transpose(1, 0, 2))

    shof = ((np.arange(cfg.n_cand, dtype=np.float32) // 8) * cfg.block
            + 1.0).astype(np.float32)
    common = {
        "kpk": kpk,
        "vals": np.ascontiguousarray(values).astype(ml_dtypes.bfloat16),
        "wqT": wqp,
        "wg": wgp,
        "wo": wop,
        "shof": shof,
    }
    in_maps = []
    for c in range(N_CORES):
        xc = xf[c * cfg.T:(c + 1) * cfg.T]        # [T, dm]
        xTc = np.ascontiguousarray(xc.T)          # [dm, T]
        xp = np.ascontiguousarray(
            xTc.reshape(n_dm, 128, cfg.T).transpose(1, 0, 2))
        m = dict(common)
        m["xT"] = xp
        in_maps.append(m)
    return in_maps


def kernel(x, keys, values, w_q, w_gate, w_out):
    cfg = FULL
    if "nc" not in _CACHE:
        _CACHE["nc"] = build(cfg)
    nc = _CACHE["nc"]
    x = np.asarray(x)
    in_maps = _prep(x, np.asarray(keys), np.asarray(values),
                    np.asarray(w_q), np.asarray(w_gate), np.asarray(w_out),
                    cfg)
    trace = os.environ.get("KERNEL_TRACE", "0") == "1"
    if trace:
        try:
            import ntff_shim
            ntff_shim.install()
        except Exception:
            pass
    res = run_bass_kernel_spmd(nc, in_maps, list(range(N_CORES)), trace=trace)
    if trace:
        _CACHE["exec_time_ns"] = res.exec_time_ns
    outs = [res.results[c]["out"] for c in range(N_CORES)]
    B, S, D = x.shape
    return np.concatenate(outs, axis=0).reshape(B, S, D)
